# revision 27
# baseline (speedup 1.0000x reference)
"""Multi-head causal self-attention (B=2, T=2048, C=768, H=12, D=64) on 8
Trainium2 NeuronCores.

Sharding: 24 (batch, head) units -> 3 heads per core; cores 0-3 take batch 0,
cores 4-7 take batch 1. Each core computes q/k/v projections for its 3 heads,
flash-style causal attention fully on-chip (no T x T tensor ever touches HBM),
and a partial output projection with its 192-row slice of Wproj. The host sums
the 4 partial projections per batch.

Device design notes:
  - Projections (x @ W) run in float32r (full PE rate at K=128, ~1e-4 rel
    err); the attention stage (S^T and P.T V matmuls) runs in bf16, which is
    insensitive to contraction size and SBUF port pressure and halves SBUF
    traffic. Accumulation is always fp32 in PSUM.
  - x arrives host-pre-transposed, packed per (chunk, k-tile) so each DMA is
    one 2D-contiguous transfer and compute starts after the first ~1.5MB.
  - q^T/k^T live in [64, T]-per-head bf16 tiles; S^T = K^T.T @ Q^T is
    computed transposed [tk, tq] so exp(S^T) feeds the P.T @ V matmul
    directly - no on-chip transposes anywhere. K^T tiles are zero-padded to
    K=128 and the zero-block position selects which half of the shared
    [q0;q1] rhs tile contributes.
  - V is augmented with a ones column per head, so the PV accumulation
    yields the softmax denominator as psum row 64 for free.
  - Causal masking: matmul columns restricted to tq >= tk-block start; the
    diagonal 128x128 sub-block gets a strictly-lower-triangular -1e30
    additive mask before exp. Chunk-major emission pipelines QKV production,
    attention, and the output projection.
  - Output is written in chunked [i, n, 128, 512] layout (contiguous DMA);
    host reassembles and reduces.
"""

import os
import sys

sys.path.insert(0, "/opt/trn_rl_repo")

import ml_dtypes
import numpy as np

import concourse.bass as bass
import concourse.tile as tile
from concourse import bacc, mybir
from concourse import bass_utils

B, T, C = 2, 2048, 768
H, D = 12, 64
N_CORES = 8
H_LOC = 3           # heads per core
DL = H_LOC * D      # 192 local head dims
TQ = 512            # tq chunk (psum bank width)
TB = 128            # tk block
NCH = T // TQ       # 4 chunks
NBL = TQ // TB      # 4 blocks per chunk
NKT = C // 128      # 6 contraction k-tiles

f32 = mybir.dt.float32
f32r = mybir.dt.float32r
bf16 = mybir.dt.bfloat16
EXP = mybir.ActivationFunctionType.Exp

LAST_RESULT = None  # test harness reads exec_time_ns from here


def _build_program(use_bias: bool):
    from contextlib import ExitStack

    nc = bacc.Bacc("TRN2", target_bir_lowering=False, debug=False,
                   num_devices=N_CORES)

    xt_d = nc.dram_tensor("xt", [NCH, 128, NKT, TQ], bf16, kind="ExternalInput").ap()
    xt1_d = nc.dram_tensor("xt1", [1, TQ], bf16, kind="ExternalInput").ap()
    wqk_d = nc.dram_tensor("wqk", [128, 7, 2 * DL], bf16, kind="ExternalInput").ap()
    wv_d = nc.dram_tensor("wv", [128, 7, 256], bf16, kind="ExternalInput").ap()
    wp_d = nc.dram_tensor("wp", [2, 128, C], bf16, kind="ExternalInput").ap()
    out_d = nc.dram_tensor("outc", [NCH, C // 128, 128, TQ], f32,
                           kind="ExternalOutput").ap()

    with tile.TileContext(nc) as tc, ExitStack() as ctx:
        cpool = ctx.enter_context(tc.tile_pool(name="const", bufs=1))
        wpool = ctx.enter_context(tc.tile_pool(name="w", bufs=1))
        xpool = ctx.enter_context(tc.tile_pool(name="x", bufs=1))
        qkpool = ctx.enter_context(tc.tile_pool(name="qk", bufs=1))

        ones_b = cpool.tile([1, D], bf16)
        nc.vector.memset(ones_b[:], 1.0)

        # --- input loads. Separate tiles per slot (Tile tracks deps per
        # tile); DMA order: wqk, chunk-0 x, wv, remaining x chunks.
        xt = [[None] * NKT for _ in range(NCH)]
        for j in range(NKT):
            t_ = xpool.tile([128, TQ], bf16, tag=f"xt0_{j}", name=f"xt0_{j}")
            nc.sync.dma_start(t_[:], xt_d[0, :, j, :])
            xt[0][j] = t_
        wqk = []
        for j in range(7):
            t_ = wpool.tile([128, 2 * DL], bf16, tag=f"wqk{j}", name=f"wqk{j}")
            nc.sync.dma_start(t_[:], wqk_d[:, j, :])
            wqk.append(t_)
        wv = []
        for j in range(7):
            t_ = wpool.tile([128, 256], bf16, tag=f"wv{j}", name=f"wv{j}")
            nc.sync.dma_start(t_[:], wv_d[:, j, :])
            wv.append(t_)
        for t in range(1, NCH):
            for j in range(NKT):
                t_ = xpool.tile([128, TQ], bf16, tag=f"xt{t}_{j}", name=f"xt{t}_{j}")
                nc.sync.dma_start(t_[:], xt_d[t, :, j, :])
                xt[t][j] = t_
        if use_bias:
            xt1 = xpool.tile([1, TQ], bf16)
            nc.sync.dma_start(xt1[:], xt1_d[:])
        wp = wpool.tile([128, C], bf16)
        nc.sync.dma_start(wp[:], wp_d[0])
        wp2 = wpool.tile([128, C], bf16)     # rows 64-127 are zeros (host pads)
        nc.sync.dma_start(wp2[:], wp_d[1])

        # Attention-stage tiles (bf16). S^T contraction is zero-padded to
        # K=128; each head's K^T has the other 64 rows zeroed, and the
        # zero-block position selects which half of the shared [q0;q1] rhs
        # tile contributes.
        zf = cpool.tile([64, TQ], bf16)
        nc.vector.memset(zf[:], 0.0)
        qTA, qTC, kT0, kT1, kT2 = [], [], [], [], []
        for t in range(NCH):
            qTA.append(qkpool.tile([128, TQ], bf16, tag=f"qTA{t}", name=f"qTA{t}"))  # [q0 ; q1]
            qTC.append(qkpool.tile([128, TQ], bf16, tag=f"qTC{t}", name=f"qTC{t}"))  # [q2 ; q0]
            kT0.append(qkpool.tile([128, TQ], bf16, tag=f"kT0{t}", name=f"kT0{t}"))  # [k0 ; 0]
            kT1.append(qkpool.tile([128, TQ], bf16, tag=f"kT1{t}", name=f"kT1{t}"))  # [0 ; k1]
            kT2.append(qkpool.tile([128, TQ], bf16, tag=f"kT2{t}", name=f"kT2{t}"))  # [k2 ; 0]
            nc.vector.tensor_copy(kT0[t][64:128, :], zf[:])
            nc.vector.tensor_copy(kT1[t][0:64, :], zf[:])
            nc.vector.tensor_copy(kT2[t][64:128, :], zf[:])
        v_sb = [qkpool.tile([128, 3 * (D + 1)], bf16, tag=f"v{t}", name=f"v{t}")
                for t in range(T // TB)]
        # per-chunk normalized-O^T tiles (per-tile deps: deferred proj of
        # chunk t-1 must not wait on chunk t's normalize)
        prhs0 = [qkpool.tile([128, TQ], bf16, tag=f"prhs0{t}", name=f"prhs0{t}")
                 for t in range(NCH)]      # heads 0,1
        prhs1 = [qkpool.tile([128, TQ], bf16, tag=f"prhs1{t}", name=f"prhs1{t}")
                 for t in range(NCH)]      # head 2 (rows 64+ zero)
        for t in range(NCH):
            nc.vector.tensor_copy(prhs1[t][64:128, :], zf[:])

        qT = [qTA, qTA, qTC]        # zero rows in kT select the head half
        kT = [kT0, kT1, kT2]

        # PSUM budget (8 banks): s 4 + po 2 + mix 2. "mix" is shared by
        # qkv-production psums, the denominator broadcast, and the
        # projection psums (ring cycles in program order).
        s_ps = ctx.enter_context(tc.tile_pool(name="s_ps", bufs=2, space="PSUM"))
        po_ps = ctx.enter_context(tc.tile_pool(name="po_ps", bufs=2, space="PSUM"))
        mix_ps = ctx.enter_context(tc.tile_pool(name="mix_ps", bufs=2, space="PSUM"))
        pt_p = ctx.enter_context(tc.tile_pool(name="pt_p", bufs=6))
        nrm = ctx.enter_context(tc.tile_pool(name="nrm", bufs=2))
        outp = ctx.enter_context(tc.tile_pool(name="outp", bufs=3))

        # Warm-up: the PE is idle during the input-DMA head and would start
        # cold (HAM 1.2GHz). Dummy matmuls on a zeroed tile (no DMA deps)
        # keep the clock warm until real work arrives.
        for w in range(48):
            wps = mix_ps.tile([128, TQ], f32, tag="mix", name=f"warm{w}")
            nc.tensor.matmul(wps[:], zf[:, 0:128], zf[:], start=True, stop=True)

        def emit_qk_group(t, m):
            # chunk t of q^T/k^T; M-tiles: [q0|q1], [k0|k1], [q2|k2]
            ps = mix_ps.tile([128, TQ], f32, tag="mix", name=f"ps_{t}_{m}")
            for j in range(NKT):
                nc.tensor.matmul(
                    ps[:],
                    wqk[j][:, 128 * m : 128 * (m + 1)],
                    xt[t][j][:],
                    start=(j == 0),
                    stop=(j == NKT - 1 and not use_bias),
                )
            if use_bias:
                nc.tensor.matmul(
                    ps[:], wqk[6][0:1, 128 * m : 128 * (m + 1)],
                    xt1[:], start=False, stop=True,
                )
            if m == 0:
                nc.vector.tensor_copy(qTA[t][:], ps[:])                # q0;q1
                nc.vector.tensor_copy(qTC[t][64:128, :], ps[0:64, :])  # q0
            elif m == 1:
                nc.vector.tensor_copy(kT0[t][0:64, :], ps[0:64, :])    # k0
                nc.vector.tensor_copy(kT1[t][64:128, :], ps[64:128, :])  # k1
            else:
                nc.vector.tensor_copy(qTC[t][0:64, :], ps[0:64, :])    # q2
                nc.vector.tensor_copy(kT2[t][0:64, :], ps[64:128, :])  # k2

        def emit_v_group(t, tb):
            # v block tb in [t, d] layout; wv interleaves [v_h | ones] per
            # head. Without bias the ones columns are memset directly.
            psv = mix_ps.tile([128, TQ], f32, tag="mix", name=f"psv_{tb}")
            for j in range(NKT):
                nc.tensor.matmul(
                    psv[0:128, 0:256],
                    xt[t][j][:, TB * (tb % NBL) : TB * (tb % NBL + 1)],
                    wv[j][:],
                    start=(j == 0), stop=(j == NKT - 1 and not use_bias),
                )
            if use_bias:
                nc.tensor.matmul(
                    psv[0:128, 0:256],
                    xt1[0:1, 0:TB],
                    wv[6][0:1, :],
                    start=False, stop=True,
                )
            nc.vector.tensor_copy(v_sb[tb][:], psv[:, 0 : 3 * (D + 1)])
            if not use_bias:
                for h in range(H_LOC):
                    c1 = (D + 1) * h + D
                    nc.vector.memset(v_sb[tb][:, c1 : c1 + 1], 1.0)

        def emit_attn_head(i, h, filler=None):
            nblk = NBL * (i + 1)
            po = po_ps.tile([D + 1, TQ], f32, tag="po", name=f"po_{i}_{h}")
            for p in range(nblk // 2):
                if filler is not None and p > 0:
                    filler()
                # two tk-blocks share a [128, 1024] psum tile -> one exp
                ps2 = s_ps.tile([128, 2 * TQ], f32, tag="s", name=f"s_{i}_{h}_{p}")
                c0s = []
                for half in range(2):
                    Bq = 2 * p + half
                    j = Bq - NBL * i
                    c0 = 0 if j < 0 else TB * j
                    c0s.append(c0)
                    off = TQ * half
                    nc.tensor.matmul(
                        ps2[:, off + c0 : off + TQ],
                        kT[h][Bq // NBL][:, TB * (Bq % NBL) : TB * (Bq % NBL + 1)],
                        qT[h][i][:, c0:TQ],
                        start=True, stop=True,
                    )
                pt = pt_p.tile([128, 2 * TQ], bf16, tag="pt", name=f"pt_{i}_{h}_{p}")
                nc.scalar.activation(pt[:, c0s[0] :], ps2[:, c0s[0] :], EXP)
                for half in range(2):
                    Bq = 2 * p + half
                    j = Bq - NBL * i
                    c0 = c0s[half]
                    off = TQ * half
                    if j >= 0:
                        # causal: zero P^T where tk > tq (on idle GpSimd)
                        nc.gpsimd.affine_select(
                            pt[:, off + TB * j : off + TB * (j + 1)],
                            pt[:, off + TB * j : off + TB * (j + 1)],
                            pattern=[[1, TB]],
                            compare_op=mybir.AluOpType.is_ge,
                            fill=0.0,
                            base=0,
                            channel_multiplier=-1,
                        )
                    nc.tensor.matmul(
                        po[:, c0:TQ],
                        v_sb[Bq][:, (D + 1) * h : (D + 1) * (h + 1)],
                        pt[:, off + c0 : off + TQ],
                        start=(Bq == 0), stop=(Bq == nblk - 1),
                    )
            # normalize: row D of po is the softmax denominator. Broadcast it
            # across 64 partitions via a rank-1 matmul, take a fast approx
            # reciprocal, multiply into O^T.
            d_sb = nrm.tile([1, TQ], bf16, tag="d", name=f"d_{i}_{h}")
            nc.vector.tensor_copy(d_sb[:], po[D : D + 1, :])
            pb = mix_ps.tile([128, TQ], f32, tag="mix", name=f"pb_{i}_{h}")
            nc.tensor.matmul(pb[0:D, :], ones_b[:], d_sb[:],
                             start=True, stop=True)
            rb = nrm.tile([D, TQ], f32, tag="rb", name=f"rb_{i}_{h}")
            nc.vector.reciprocal_approx_fast(rb[:], pb[0:D, :])
            dst = (prhs0[i][64 * h : 64 * (h + 1), :]
                   if h < 2 else prhs1[i][0:64, :])
            nc.vector.tensor_mul(dst, po[0:D, :], rb[:])

        def emit_proj(i, n):
            # projection chunk (wp2/prhs1 zero-padded to K=128)
            pp = mix_ps.tile([128, TQ], f32, tag="mix", name=f"pp_{i}_{n}")
            nc.tensor.matmul(pp[:], wp[:, 128 * n : 128 * (n + 1)],
                             prhs0[i][:], start=True, stop=False)
            nc.tensor.matmul(pp[:], wp2[:, 128 * n : 128 * (n + 1)],
                             prhs1[i][:], start=False, stop=True)
            osb = outp.tile([128, TQ], f32, tag="out", name=f"osb_{i}_{n}")
            nc.vector.tensor_copy(osb[:], pp[:])
            nc.sync.dma_start(out_d[i, n], osb[:])

        # Chunk-major pipeline. The attention inner loop is ACT(exp)-bound,
        # so independent PE work - chunk t+1's production matmuls and chunk
        # t-1's projection - is woven between attention heads of chunk t:
        # the in-order PE queue then has dense work while ACT catches up
        # (keeps the PE HAM clock warm). Chunk 3's attention, the largest,
        # gets chunk 2's projection; chunk 3's projection runs at the end.
        for m in range(3):
            emit_qk_group(0, m)
        for tb in range(NBL):
            emit_v_group(0, tb)
        def run_piece(piece):
            kind, a, b = piece
            if kind == "qk":
                emit_qk_group(a, b)
            elif kind == "v":
                emit_v_group(a, b)
            else:
                emit_proj(a, b)

        for t in range(NCH):
            nxt = []
            if t + 1 < NCH:
                nxt = [("qk", t + 1, m) for m in range(3)] + \
                      [("v", t + 1, tb) for tb in range(NBL * (t + 1), NBL * (t + 2))]
            else:
                # chunk 3 has no production left; fill with chunk 2's proj
                nxt = [("proj", 2, n) for n in range(C // 128)]
            nslots = H_LOC * max(0, (t + 1) * 2 - 1)
            per_slot = -(-len(nxt) // nslots) if nslots else len(nxt)

            def filler():
                take = per_slot
                while take and nxt:
                    run_piece(nxt.pop(0))
                    take -= 1

            for h in range(H_LOC):
                emit_attn_head(t, h, filler)
                if h == H_LOC - 1:
                    while nxt:
                        run_piece(nxt.pop(0))
            if t < NCH - 2:
                for n in range(C // 128):
                    emit_proj(t, n)
        for n in range(C // 128):
            emit_proj(NCH - 1, n)

    nc.compile()
    return nc


_PROG_CACHE = {}


def kernel(x, Wqkv, bqkv, Wproj, bproj):
    global LAST_RESULT
    x = np.asarray(x, dtype=np.float32)
    Wqkv = np.asarray(Wqkv, dtype=np.float32)
    bqkv = np.asarray(bqkv, dtype=np.float32)
    Wproj = np.asarray(Wproj, dtype=np.float32)
    bproj = np.asarray(bproj, dtype=np.float32)

    Wq, Wk, Wv = Wqkv[:, 0:C], Wqkv[:, C : 2 * C], Wqkv[:, 2 * C : 3 * C]
    bq, bk, bv = bqkv[0:C], bqkv[C : 2 * C], bqkv[2 * C : 3 * C]
    scale = 1.0 / np.sqrt(D)

    use_bias = bool(np.any(bq) or np.any(bk) or np.any(bv))
    if use_bias not in _PROG_CACHE:
        _PROG_CACHE[use_bias] = _build_program(use_bias)
    nc = _PROG_CACHE[use_bias]

    in_maps = []
    for c in range(N_CORES):
        b = c // (N_CORES // B)
        g = c % (N_CORES // B)
        hs = slice(DL * g, DL * (g + 1))       # this core's head-dim rows/cols

        # x^T packed per (chunk, k-tile): [NCH, 128, NKT, TQ]
        xt = np.ascontiguousarray(
            x[b].T.reshape(NKT, 128, NCH, TQ).transpose(2, 1, 0, 3))
        xt1 = np.ones((1, TQ), np.float32)

        wq_loc = Wq[:, hs] * scale             # fold 1/sqrt(D) into q
        bq_loc = bq[hs] * scale
        wk_loc, bk_loc = Wk[:, hs], bk[hs]
        wv_loc, bv_loc = Wv[:, hs], bv[hs]

        wqk = np.zeros((C + 128, 2 * DL), np.float32)   # 7 k-tiles of 128
        wqk[0:C, 0:128] = wq_loc[:, 0:128]
        wqk[C, 0:128] = bq_loc[0:128]
        wqk[0:C, 128:256] = wk_loc[:, 0:128]
        wqk[C, 128:256] = bk_loc[0:128]
        wqk[0:C, 256:320] = wq_loc[:, 128:192]
        wqk[C, 256:320] = bq_loc[128:192]
        wqk[0:C, 320:384] = wk_loc[:, 128:192]
        wqk[C, 320:384] = bk_loc[128:192]
        wqk = np.ascontiguousarray(wqk.reshape(7, 128, 2 * DL).transpose(1, 0, 2))

        wv_pad = np.zeros((C + 128, 256), np.float32)
        for h in range(H_LOC):
            c0 = (D + 1) * h
            wv_pad[0:C, c0 : c0 + D] = wv_loc[:, D * h : D * (h + 1)]
            wv_pad[C, c0 : c0 + D] = bv_loc[D * h : D * (h + 1)]
            wv_pad[C, c0 + D] = 1.0            # ones column -> softmax denom
        wv_pad = np.ascontiguousarray(wv_pad.reshape(7, 128, 256).transpose(1, 0, 2))

        wp = np.zeros((2, 128, C), np.float32)
        wp[0] = Wproj[DL * g : DL * g + 128, :]  # cast to bf16 below
        wp[1, 0:64] = Wproj[DL * g + 128 : DL * (g + 1), :]

        bf = ml_dtypes.bfloat16
        in_maps.append({"xt": xt.astype(bf), "xt1": xt1.astype(bf),
                        "wqk": wqk.astype(bf), "wv": wv_pad.astype(bf), "wp": wp.astype(bf)})

    res = bass_utils.run_bass_kernel_spmd(nc, in_maps, core_ids=list(range(N_CORES)))
    LAST_RESULT = res

    out = np.zeros((B, T, C), np.float32)
    for c in range(N_CORES):
        b = c // (N_CORES // B)
        # outc [i, n, 128, 512] -> [C, T] -> [T, C]
        outT = res.results[c]["outc"].transpose(1, 2, 0, 3).reshape(C, T)
        out[b] += outT.T
    return out + bproj


if __name__ == "__main__":
    rng = np.random.default_rng(0)
    s = 1.0 / np.sqrt(C)
    ins = {
        "x": rng.standard_normal((B, T, C), dtype=np.float32),
        "Wqkv": rng.standard_normal((C, 3 * C), dtype=np.float32) * s,
        "bqkv": np.zeros(3 * C, np.float32),
        "Wproj": rng.standard_normal((C, C), dtype=np.float32) * s,
        "bproj": np.zeros(C, np.float32),
    }
    out = kernel(**ins)
    print("out", out.shape, out.dtype, float(np.abs(out).max()))


# revision 28
# speedup vs baseline: 1.1064x; 1.1064x over previous
"""Multi-head causal self-attention (B=2, T=2048, C=768, H=12, D=64) on 8
Trainium2 NeuronCores.

Sharding: 24 (batch, head) units -> 3 heads per core; cores 0-3 take batch 0,
cores 4-7 take batch 1. Each core computes q/k/v projections for its 3 heads,
flash-style causal attention fully on-chip (no T x T tensor ever touches HBM),
and a partial output projection with its 192-row slice of Wproj. The host sums
the 4 partial projections per batch.

Device design notes:
  - Projections (x @ W) run in float32r (full PE rate at K=128, ~1e-4 rel
    err); the attention stage (S^T and P.T V matmuls) runs in bf16, which is
    insensitive to contraction size and SBUF port pressure and halves SBUF
    traffic. Accumulation is always fp32 in PSUM.
  - x arrives host-pre-transposed, packed per (chunk, k-tile) so each DMA is
    one 2D-contiguous transfer and compute starts after the first ~1.5MB.
  - q^T/k^T live in [64, T]-per-head bf16 tiles; S^T = K^T.T @ Q^T is
    computed transposed [tk, tq] so exp(S^T) feeds the P.T @ V matmul
    directly - no on-chip transposes anywhere. K^T tiles are zero-padded to
    K=128 and the zero-block position selects which half of the shared
    [q0;q1] rhs tile contributes.
  - V is augmented with a ones column per head, so the PV accumulation
    yields the softmax denominator as psum row 64 for free.
  - Causal masking: matmul columns restricted to tq >= tk-block start; the
    diagonal 128x128 sub-block gets a strictly-lower-triangular -1e30
    additive mask before exp. Chunk-major emission pipelines QKV production,
    attention, and the output projection.
  - Output is written in chunked [i, n, 128, 512] layout (contiguous DMA);
    host reassembles and reduces.
"""

import os
import sys

sys.path.insert(0, "/opt/trn_rl_repo")

import ml_dtypes
import numpy as np

import concourse.bass as bass
import concourse.tile as tile
from concourse import bacc, mybir
from concourse import bass_utils

B, T, C = 2, 2048, 768
H, D = 12, 64
N_CORES = 8
H_LOC = 3           # heads per core
DL = H_LOC * D      # 192 local head dims
TQ = 512            # tq chunk (psum bank width)
TB = 128            # tk block
NCH = T // TQ       # 4 chunks
NBL = TQ // TB      # 4 blocks per chunk
NKT = C // 128      # 6 contraction k-tiles

f32 = mybir.dt.float32
f32r = mybir.dt.float32r
bf16 = mybir.dt.bfloat16
EXP = mybir.ActivationFunctionType.Exp

LAST_RESULT = None  # test harness reads exec_time_ns from here


def _build_program(use_bias: bool):
    from contextlib import ExitStack

    nc = bacc.Bacc("TRN2", target_bir_lowering=False, debug=False,
                   num_devices=N_CORES)

    xt_d = nc.dram_tensor("xt", [NCH, 128, NKT, TQ], bf16, kind="ExternalInput").ap()
    xt1_d = nc.dram_tensor("xt1", [1, TQ], bf16, kind="ExternalInput").ap()
    wqk_d = nc.dram_tensor("wqk", [128, 7, 2 * DL], bf16, kind="ExternalInput").ap()
    wv_d = nc.dram_tensor("wv", [128, 7, 256], bf16, kind="ExternalInput").ap()
    wp_d = nc.dram_tensor("wp", [2, 128, C], bf16, kind="ExternalInput").ap()
    out_d = nc.dram_tensor("outc", [NCH, C // 128, 128, TQ], f32,
                           kind="ExternalOutput").ap()

    with tile.TileContext(nc) as tc, ExitStack() as ctx:
        cpool = ctx.enter_context(tc.tile_pool(name="const", bufs=1))
        wpool = ctx.enter_context(tc.tile_pool(name="w", bufs=1))
        xpool = ctx.enter_context(tc.tile_pool(name="x", bufs=1))
        qkpool = ctx.enter_context(tc.tile_pool(name="qk", bufs=1))

        ones_b = cpool.tile([1, D], bf16)
        nc.vector.memset(ones_b[:], 1.0)

        # --- input loads. Separate tiles per slot (Tile tracks deps per
        # tile); DMA order: wqk, chunk-0 x, wv, remaining x chunks.
        xt = [[None] * NKT for _ in range(NCH)]
        for j in range(NKT):
            t_ = xpool.tile([128, TQ], bf16, tag=f"xt0_{j}", name=f"xt0_{j}")
            nc.sync.dma_start(t_[:], xt_d[0, :, j, :])
            xt[0][j] = t_
        wqk = []
        for j in range(7):
            t_ = wpool.tile([128, 2 * DL], bf16, tag=f"wqk{j}", name=f"wqk{j}")
            nc.sync.dma_start(t_[:], wqk_d[:, j, :])
            wqk.append(t_)
        wv = []
        for j in range(7):
            t_ = wpool.tile([128, 256], bf16, tag=f"wv{j}", name=f"wv{j}")
            nc.sync.dma_start(t_[:], wv_d[:, j, :])
            wv.append(t_)
        for t in range(1, NCH):
            for j in range(NKT):
                t_ = xpool.tile([128, TQ], bf16, tag=f"xt{t}_{j}", name=f"xt{t}_{j}")
                nc.sync.dma_start(t_[:], xt_d[t, :, j, :])
                xt[t][j] = t_
        if use_bias:
            xt1 = xpool.tile([1, TQ], bf16)
            nc.sync.dma_start(xt1[:], xt1_d[:])
        wp = wpool.tile([128, C], bf16)
        nc.sync.dma_start(wp[:], wp_d[0])
        wp2 = wpool.tile([128, C], bf16)     # rows 64-127 are zeros (host pads)
        nc.sync.dma_start(wp2[:], wp_d[1])

        # Attention-stage tiles (bf16). S^T contraction is zero-padded to
        # K=128; each head's K^T has the other 64 rows zeroed, and the
        # zero-block position selects which half of the shared [q0;q1] rhs
        # tile contributes.
        zf = cpool.tile([64, TQ], bf16)
        nc.vector.memset(zf[:], 0.0)
        qTA, qTC, kT0, kT1, kT2 = [], [], [], [], []
        for t in range(NCH):
            qTA.append(qkpool.tile([128, TQ], bf16, tag=f"qTA{t}", name=f"qTA{t}"))  # [q0 ; q1]
            qTC.append(qkpool.tile([128, TQ], bf16, tag=f"qTC{t}", name=f"qTC{t}"))  # [q2 ; q0]
            kT0.append(qkpool.tile([128, TQ], bf16, tag=f"kT0{t}", name=f"kT0{t}"))  # [k0 ; 0]
            kT1.append(qkpool.tile([128, TQ], bf16, tag=f"kT1{t}", name=f"kT1{t}"))  # [0 ; k1]
            kT2.append(qkpool.tile([128, TQ], bf16, tag=f"kT2{t}", name=f"kT2{t}"))  # [k2 ; 0]
            nc.vector.tensor_copy(kT0[t][64:128, :], zf[:])
            nc.vector.tensor_copy(kT1[t][0:64, :], zf[:])
            nc.vector.tensor_copy(kT2[t][64:128, :], zf[:])
        v_sb = [qkpool.tile([128, 3 * (D + 1)], bf16, tag=f"v{t}", name=f"v{t}")
                for t in range(T // TB)]
        # per-chunk normalized-O^T tiles (per-tile deps: deferred proj of
        # chunk t-1 must not wait on chunk t's normalize)
        prhs0 = [qkpool.tile([128, TQ], bf16, tag=f"prhs0{t}", name=f"prhs0{t}")
                 for t in range(NCH)]      # heads 0,1
        prhs1 = [qkpool.tile([128, TQ], bf16, tag=f"prhs1{t}", name=f"prhs1{t}")
                 for t in range(NCH)]      # head 2 (rows 64+ zero)
        for t in range(NCH):
            nc.vector.tensor_copy(prhs1[t][64:128, :], zf[:])

        qT = [qTA, qTA, qTC]        # zero rows in kT select the head half
        kT = [kT0, kT1, kT2]

        # PSUM budget (8 banks): s 4 + po 2 + mix 2. "mix" is shared by
        # qkv-production psums, the denominator broadcast, and the
        # projection psums (ring cycles in program order).
        s_ps = ctx.enter_context(tc.tile_pool(name="s_ps", bufs=2, space="PSUM"))
        po_ps = ctx.enter_context(tc.tile_pool(name="po_ps", bufs=2, space="PSUM"))
        mix_ps = ctx.enter_context(tc.tile_pool(name="mix_ps", bufs=2, space="PSUM"))
        pt_p = ctx.enter_context(tc.tile_pool(name="pt_p", bufs=6))
        nrm = ctx.enter_context(tc.tile_pool(name="nrm", bufs=2))
        outp = ctx.enter_context(tc.tile_pool(name="outp", bufs=3))

        def emit_qk_group(t, m):
            # chunk t of q^T/k^T; M-tiles: [q0|q1], [k0|k1], [q2|k2]
            ps = mix_ps.tile([128, TQ], f32, tag="mix", name=f"ps_{t}_{m}")
            for j in range(NKT):
                nc.tensor.matmul(
                    ps[:],
                    wqk[j][:, 128 * m : 128 * (m + 1)],
                    xt[t][j][:],
                    start=(j == 0),
                    stop=(j == NKT - 1 and not use_bias),
                )
            if use_bias:
                nc.tensor.matmul(
                    ps[:], wqk[6][0:1, 128 * m : 128 * (m + 1)],
                    xt1[:], start=False, stop=True,
                )
            if m == 0:
                nc.vector.tensor_copy(qTA[t][:], ps[:])                # q0;q1
                nc.vector.tensor_copy(qTC[t][64:128, :], ps[0:64, :])  # q0
            elif m == 1:
                nc.vector.tensor_copy(kT0[t][0:64, :], ps[0:64, :])    # k0
                nc.vector.tensor_copy(kT1[t][64:128, :], ps[64:128, :])  # k1
            else:
                nc.vector.tensor_copy(qTC[t][0:64, :], ps[0:64, :])    # q2
                nc.vector.tensor_copy(kT2[t][0:64, :], ps[64:128, :])  # k2

        def emit_v_group(t, tb):
            # v block tb in [t, d] layout; wv interleaves [v_h | ones] per
            # head. Without bias the ones columns are memset directly.
            psv = mix_ps.tile([128, TQ], f32, tag="mix", name=f"psv_{tb}")
            for j in range(NKT):
                nc.tensor.matmul(
                    psv[0:128, 0:256],
                    xt[t][j][:, TB * (tb % NBL) : TB * (tb % NBL + 1)],
                    wv[j][:],
                    start=(j == 0), stop=(j == NKT - 1 and not use_bias),
                )
            if use_bias:
                nc.tensor.matmul(
                    psv[0:128, 0:256],
                    xt1[0:1, 0:TB],
                    wv[6][0:1, :],
                    start=False, stop=True,
                )
            nc.vector.tensor_copy(v_sb[tb][:], psv[:, 0 : 3 * (D + 1)])
            if not use_bias:
                for h in range(H_LOC):
                    c1 = (D + 1) * h + D
                    nc.vector.memset(v_sb[tb][:, c1 : c1 + 1], 1.0)

        def emit_attn_head(i, h, filler=None):
            nblk = NBL * (i + 1)
            po = po_ps.tile([D + 1, TQ], f32, tag="po", name=f"po_{i}_{h}")
            for p in range(nblk // 2):
                if filler is not None and p > 0:
                    filler()
                # two tk-blocks share a [128, 1024] psum tile -> one exp
                ps2 = s_ps.tile([128, 2 * TQ], f32, tag="s", name=f"s_{i}_{h}_{p}")
                c0s = []
                for half in range(2):
                    Bq = 2 * p + half
                    j = Bq - NBL * i
                    c0 = 0 if j < 0 else TB * j
                    c0s.append(c0)
                    off = TQ * half
                    nc.tensor.matmul(
                        ps2[:, off + c0 : off + TQ],
                        kT[h][Bq // NBL][:, TB * (Bq % NBL) : TB * (Bq % NBL + 1)],
                        qT[h][i][:, c0:TQ],
                        start=True, stop=True,
                    )
                pt = pt_p.tile([128, 2 * TQ], bf16, tag="pt", name=f"pt_{i}_{h}_{p}")
                nc.scalar.activation(pt[:, c0s[0] :], ps2[:, c0s[0] :], EXP)
                for half in range(2):
                    Bq = 2 * p + half
                    j = Bq - NBL * i
                    c0 = c0s[half]
                    off = TQ * half
                    if j >= 0:
                        # causal: zero P^T where tk > tq (on idle GpSimd)
                        nc.gpsimd.affine_select(
                            pt[:, off + TB * j : off + TB * (j + 1)],
                            pt[:, off + TB * j : off + TB * (j + 1)],
                            pattern=[[1, TB]],
                            compare_op=mybir.AluOpType.is_ge,
                            fill=0.0,
                            base=0,
                            channel_multiplier=-1,
                        )
                    nc.tensor.matmul(
                        po[:, c0:TQ],
                        v_sb[Bq][:, (D + 1) * h : (D + 1) * (h + 1)],
                        pt[:, off + c0 : off + TQ],
                        start=(Bq == 0), stop=(Bq == nblk - 1),
                    )
            # normalize: row D of po is the softmax denominator. Broadcast it
            # across 64 partitions via a rank-1 matmul, take a fast approx
            # reciprocal, multiply into O^T.
            d_sb = nrm.tile([1, TQ], bf16, tag="d", name=f"d_{i}_{h}")
            nc.vector.tensor_copy(d_sb[:], po[D : D + 1, :])
            pb = mix_ps.tile([128, TQ], f32, tag="mix", name=f"pb_{i}_{h}")
            nc.tensor.matmul(pb[0:D, :], ones_b[:], d_sb[:],
                             start=True, stop=True)
            rb = nrm.tile([D, TQ], f32, tag="rb", name=f"rb_{i}_{h}")
            nc.vector.reciprocal_approx_fast(rb[:], pb[0:D, :])
            dst = (prhs0[i][64 * h : 64 * (h + 1), :]
                   if h < 2 else prhs1[i][0:64, :])
            nc.vector.tensor_mul(dst, po[0:D, :], rb[:])

        def emit_proj(i, n):
            # projection chunk (wp2/prhs1 zero-padded to K=128)
            pp = mix_ps.tile([128, TQ], f32, tag="mix", name=f"pp_{i}_{n}")
            nc.tensor.matmul(pp[:], wp[:, 128 * n : 128 * (n + 1)],
                             prhs0[i][:], start=True, stop=False)
            nc.tensor.matmul(pp[:], wp2[:, 128 * n : 128 * (n + 1)],
                             prhs1[i][:], start=False, stop=True)
            osb = outp.tile([128, TQ], f32, tag="out", name=f"osb_{i}_{n}")
            nc.vector.tensor_copy(osb[:], pp[:])
            nc.sync.dma_start(out_d[i, n], osb[:])

        # Chunk-major pipeline. The attention inner loop is ACT(exp)-bound,
        # so independent PE work - chunk t+1's production matmuls and chunk
        # t-1's projection - is woven between attention heads of chunk t:
        # the in-order PE queue then has dense work while ACT catches up
        # (keeps the PE HAM clock warm). Chunk 3's attention, the largest,
        # gets chunk 2's projection; chunk 3's projection runs at the end.
        for m in range(3):
            emit_qk_group(0, m)
        for tb in range(NBL):
            emit_v_group(0, tb)
        def run_piece(piece):
            kind, a, b = piece
            if kind == "qk":
                emit_qk_group(a, b)
            elif kind == "v":
                emit_v_group(a, b)
            else:
                emit_proj(a, b)

        for t in range(NCH):
            nxt = []
            if t + 1 < NCH:
                nxt = [("qk", t + 1, m) for m in range(3)] + \
                      [("v", t + 1, tb) for tb in range(NBL * (t + 1), NBL * (t + 2))]
            else:
                # chunk 3 has no production left; fill with chunk 2's proj
                nxt = [("proj", 2, n) for n in range(C // 128)]
            nslots = H_LOC * max(0, (t + 1) * 2 - 1)
            per_slot = -(-len(nxt) // nslots) if nslots else len(nxt)

            def filler():
                take = per_slot
                while take and nxt:
                    run_piece(nxt.pop(0))
                    take -= 1

            for h in range(H_LOC):
                emit_attn_head(t, h, filler)
                if h == H_LOC - 1:
                    while nxt:
                        run_piece(nxt.pop(0))
            if t < NCH - 2:
                for n in range(C // 128):
                    emit_proj(t, n)
        for n in range(C // 128):
            emit_proj(NCH - 1, n)

    nc.compile()
    return nc


_PROG_CACHE = {}


def kernel(x, Wqkv, bqkv, Wproj, bproj):
    global LAST_RESULT
    x = np.asarray(x, dtype=np.float32)
    Wqkv = np.asarray(Wqkv, dtype=np.float32)
    bqkv = np.asarray(bqkv, dtype=np.float32)
    Wproj = np.asarray(Wproj, dtype=np.float32)
    bproj = np.asarray(bproj, dtype=np.float32)

    Wq, Wk, Wv = Wqkv[:, 0:C], Wqkv[:, C : 2 * C], Wqkv[:, 2 * C : 3 * C]
    bq, bk, bv = bqkv[0:C], bqkv[C : 2 * C], bqkv[2 * C : 3 * C]
    scale = 1.0 / np.sqrt(D)

    use_bias = bool(np.any(bq) or np.any(bk) or np.any(bv))
    if use_bias not in _PROG_CACHE:
        _PROG_CACHE[use_bias] = _build_program(use_bias)
    nc = _PROG_CACHE[use_bias]

    in_maps = []
    for c in range(N_CORES):
        b = c // (N_CORES // B)
        g = c % (N_CORES // B)
        hs = slice(DL * g, DL * (g + 1))       # this core's head-dim rows/cols

        # x^T packed per (chunk, k-tile): [NCH, 128, NKT, TQ]
        xt = np.ascontiguousarray(
            x[b].T.reshape(NKT, 128, NCH, TQ).transpose(2, 1, 0, 3))
        xt1 = np.ones((1, TQ), np.float32)

        wq_loc = Wq[:, hs] * scale             # fold 1/sqrt(D) into q
        bq_loc = bq[hs] * scale
        wk_loc, bk_loc = Wk[:, hs], bk[hs]
        wv_loc, bv_loc = Wv[:, hs], bv[hs]

        wqk = np.zeros((C + 128, 2 * DL), np.float32)   # 7 k-tiles of 128
        wqk[0:C, 0:128] = wq_loc[:, 0:128]
        wqk[C, 0:128] = bq_loc[0:128]
        wqk[0:C, 128:256] = wk_loc[:, 0:128]
        wqk[C, 128:256] = bk_loc[0:128]
        wqk[0:C, 256:320] = wq_loc[:, 128:192]
        wqk[C, 256:320] = bq_loc[128:192]
        wqk[0:C, 320:384] = wk_loc[:, 128:192]
        wqk[C, 320:384] = bk_loc[128:192]
        wqk = np.ascontiguousarray(wqk.reshape(7, 128, 2 * DL).transpose(1, 0, 2))

        wv_pad = np.zeros((C + 128, 256), np.float32)
        for h in range(H_LOC):
            c0 = (D + 1) * h
            wv_pad[0:C, c0 : c0 + D] = wv_loc[:, D * h : D * (h + 1)]
            wv_pad[C, c0 : c0 + D] = bv_loc[D * h : D * (h + 1)]
            wv_pad[C, c0 + D] = 1.0            # ones column -> softmax denom
        wv_pad = np.ascontiguousarray(wv_pad.reshape(7, 128, 256).transpose(1, 0, 2))

        wp = np.zeros((2, 128, C), np.float32)
        wp[0] = Wproj[DL * g : DL * g + 128, :]  # cast to bf16 below
        wp[1, 0:64] = Wproj[DL * g + 128 : DL * (g + 1), :]

        bf = ml_dtypes.bfloat16
        in_maps.append({"xt": xt.astype(bf), "xt1": xt1.astype(bf),
                        "wqk": wqk.astype(bf), "wv": wv_pad.astype(bf), "wp": wp.astype(bf)})

    res = bass_utils.run_bass_kernel_spmd(nc, in_maps, core_ids=list(range(N_CORES)))
    LAST_RESULT = res

    out = np.zeros((B, T, C), np.float32)
    for c in range(N_CORES):
        b = c // (N_CORES // B)
        # outc [i, n, 128, 512] -> [C, T] -> [T, C]
        outT = res.results[c]["outc"].transpose(1, 2, 0, 3).reshape(C, T)
        out[b] += outT.T
    return out + bproj


if __name__ == "__main__":
    rng = np.random.default_rng(0)
    s = 1.0 / np.sqrt(C)
    ins = {
        "x": rng.standard_normal((B, T, C), dtype=np.float32),
        "Wqkv": rng.standard_normal((C, 3 * C), dtype=np.float32) * s,
        "bqkv": np.zeros(3 * C, np.float32),
        "Wproj": rng.standard_normal((C, C), dtype=np.float32) * s,
        "bproj": np.zeros(C, np.float32),
    }
    out = kernel(**ins)
    print("out", out.shape, out.dtype, float(np.abs(out).max()))
